# revision 29
# baseline (speedup 1.0000x reference)
"""Trainium2 kernel for nn_AlignedEmbeddings (bidirectional-LSTM VAE-style loss).

Strategy (8 NeuronCores, SPMD, vocab-parallel per the sharding hint):
- Host: embedding lookup + the 0.5 GFLOP *sequential* LSTM scan (256 strictly
  serial [512]x[512,2048] matvecs — latency-bound, unsuited to the 128x128 PE
  array) + the jax PRNG noise (must bit-match jax.random).
- Device (per core, ~9 GFLOP each = 99.3% of model FLOPs): U/S projections,
  softplus, z = u + s*eps formation, the gathered-logit dot products, and the
  dominant [2176,512] @ [512,8000] vocab-sharded logit matmul with fused
  bias-add + row-max (DVE tensor_tensor_reduce) + exp/row-sum (ACT accum_out)
  log-softmax partials. fW/gW vocab dim sharded 8 ways; LSTM/U/S params
  replicated.
- Host: merges per-shard (max, sumexp) into global log-softmax normalizers,
  extracts the gathered logits, and assembles -kl + term1 + term2.
"""

import os
import sys

for _p in ("/opt/trn_rl_repo", "/opt/pypackages"):
    if os.path.isdir(_p) and _p not in sys.path:
        sys.path.append(_p)

import numpy as np
import ml_dtypes

import concourse.mybir as mybir
import concourse.tile as tile
from concourse import bacc
from concourse.bass_utils import run_bass_kernel_spmd

F32 = mybir.dt.float32
BF16 = mybir.dt.bfloat16
AF = mybir.ActivationFunctionType
ALU = mybir.AluOpType
BF = ml_dtypes.bfloat16

N1, N2, E, H, V = 128, 16, 256, 512, 32000
NCORES = 8
VSH = V // NCORES            # 4000 vocab columns per core
NSEL = N1 + N2               # 144 gathered weight rows
R = N1 * (N2 + 1)            # 2176 logit rows (z1 tokens + z2 samples)
RT = R // 128                # 17 row tiles
VHALF = (2048, 1952)         # per-row-tile vocab split (PSUM-bank aligned)

_prog_cache = {}
last_result = None


# ---------------------------------------------------------------- host math
def _lstm_scan_np(x, Wih, Whh, bih, bhh):
    T = x.shape[0]
    WihT = np.ascontiguousarray(Wih.T)
    WhhT = np.ascontiguousarray(Whh.T)
    xg = x @ WihT + (bih + bhh)
    Hh = Whh.shape[1]
    h = np.zeros(Hh, np.float32)
    c = np.zeros(Hh, np.float32)
    hs = np.empty((T, Hh), np.float32)
    def sig(v):
        return 1.0 / (1.0 + np.exp(-v))
    for t in range(T):
        g = xg[t] + h @ WhhT
        i, f, gg, o = g[:Hh], g[Hh:2 * Hh], g[2 * Hh:3 * Hh], g[3 * Hh:]
        c = sig(f) * c + sig(i) * np.tanh(gg)
        h = sig(o) * np.tanh(c)
        hs[t] = h
    return hs


def _jax_noise():
    """eps1 [N1,H], eps2 [N2,N1,H] from jax.random key(42), computed on CPU."""
    try:
        import jax
        cpus = jax.devices("cpu")
        with jax.default_device(cpus[0]):
            k1, k2 = jax.random.split(jax.random.key(42))
            e1 = np.asarray(jax.random.normal(k1, (N1, H), np.float32))
            e2 = np.asarray(jax.random.normal(k2, (N2, N1, H), np.float32))
        return e1, e2
    except Exception:
        pass
    # Fallback: clean-env subprocess pinned to the CPU backend.
    import subprocess, tempfile
    code = (
        "import numpy as np, jax\n"
        "k1, k2 = jax.random.split(jax.random.key(42))\n"
        f"e1 = np.asarray(jax.random.normal(k1, ({N1}, {H}), np.float32))\n"
        f"e2 = np.asarray(jax.random.normal(k2, ({N2}, {N1}, {H}), np.float32))\n"
        "import sys; np.savez(sys.argv[1], e1=e1, e2=e2)\n"
    )
    env = dict(os.environ)
    env["JAX_PLATFORMS"] = "cpu"
    with tempfile.TemporaryDirectory() as td:
        out = os.path.join(td, "noise.npz")
        subprocess.run([sys.executable, "-c", code, out], check=True, env=env)
        z = np.load(out)
        return z["e1"], z["e2"]


# ---------------------------------------------------------------- device prog
def _build_program(stage=5):
    nc = bacc.Bacc("TRN2", target_bir_lowering=False, debug=False,
                   num_devices=NCORES)

    def din(name, shape, dt):
        return nc.dram_tensor(name, shape, dt, kind="ExternalInput").ap()

    def dout(name, shape, dt):
        return nc.dram_tensor(name, shape, dt, kind="ExternalOutput").ap()

    hT_d = din("hT", [8, 128, 128], BF16)
    UWT_d = din("UWT", [8, 128, 512], BF16)
    SWT_d = din("SWT", [8, 128, 512], BF16)
    Ub_d = din("Ub", [4, 128, 1], F32)
    Sb_d = din("Sb", [4, 128, 1], F32)
    eps_d = din("epsT", [4, 128, R], BF16)
    sel_d = din("selT", [4, 128, NSEL], BF16)
    fgW_d = din("fgWT", [4, 128, 2 * VSH], BF16)
    fgb_d = din("fgb", [1, 2 * VSH], BF16)

    uT_o = dout("uT", [4, 128, 128], F32)
    sT_o = dout("sT", [4, 128, 128], F32)
    sel1_o = dout("sel1", [128, 128], F32)
    sel2_o = dout("sel2", [16, 2048], F32)
    stats_o = dout("stats", [128, RT * 4], F32)

    from contextlib import ExitStack
    with tile.TileContext(nc) as tc, ExitStack() as stack:
        con = stack.enter_context(tc.tile_pool(name="con", bufs=1))
        # ---- input tiles (DMAs spread across engine queues)
        hT = [con.tile([128, 128], BF16, tag=f"hT{k}", name=f"hT{k}") for k in range(8)]
        UWT = [con.tile([128, 512], BF16, tag=f"UWT{k}", name=f"UWT{k}") for k in range(8)]
        SWT = [con.tile([128, 512], BF16, tag=f"SWT{k}", name=f"SWT{k}") for k in range(8)]
        Ub = [con.tile([128, 1], F32, tag=f"Ub{m}", name=f"Ub{m}") for m in range(4)]
        Sb = [con.tile([128, 1], F32, tag=f"Sb{m}", name=f"Sb{m}") for m in range(4)]
        epsT = [con.tile([128, R], BF16, tag=f"eps{c}", name=f"eps{c}") for c in range(4)]
        selT = [con.tile([128, NSEL], BF16, tag=f"sel{c}", name=f"selc{c}") for c in range(4)]
        fgWT = [con.tile([128, 2 * VSH], BF16, tag=f"fgW{c}", name=f"fgW{c}") for c in range(4)]
        fgb = con.tile([1, 2 * VSH], BF16, tag="fgb")
        ones1 = con.tile([1, 128], BF16, tag="ones1")
        nc.vector.memset(ones1[:], 1.0)

        # All DMAs on the two hardware-DGE queues (SP + ACT), urgent first.
        eng = [nc.sync, nc.scalar]
        for k in range(8):
            eng[k % 2].dma_start(hT[k][:], hT_d[k])
        for k in range(8):
            eng[k % 2].dma_start(UWT[k][:], UWT_d[k])
            eng[(k + 1) % 2].dma_start(SWT[k][:], SWT_d[k])
        for m in range(4):
            nc.sync.dma_start(Ub[m][:], Ub_d[m])
            nc.scalar.dma_start(Sb[m][:], Sb_d[m])
        for c in range(4):
            eng[c % 2].dma_start(epsT[c][:], eps_d[c])
            eng[(c + 1) % 2].dma_start(selT[c][:], sel_d[c])
        nc.sync.dma_start(fgb[:], fgb_d[:])
        # big weight tensor last (first-needed slices first), split for overlap
        for half in range(2):
            for c in range(4):
                eng[c % 2].dma_start(
                    fgWT[c][:, half * VSH:(half + 1) * VSH],
                    fgW_d[c][:, half * VSH:(half + 1) * VSH])

        work = stack.enter_context(tc.tile_pool(name="work", bufs=1))
        uTb = [work.tile([128, 128], BF16, tag=f"uTb{m}", name=f"uTb{m}") for m in range(4)]
        sTb = [work.tile([128, 128], BF16, tag=f"sTb{m}", name=f"sTb{m}") for m in range(4)]
        zTb = [work.tile([128, R], BF16, tag=f"zTb{c}", name=f"zTb{c}") for c in range(4)]
        stats = work.tile([128, RT * 4], F32, tag="stats")

        if stage < 1:
            s0 = work.tile([128, 128], F32, tag="s0", name="s0")
            nc.vector.tensor_copy(s0[:], fgWT[0][:, 0:128])
            nc.sync.dma_start(uT_o[0], s0[:])
        # ---- phase 1: uT, sT  (uT = UW @ hT + Ub ; sT = softplus(SW @ hT + Sb))
        with tc.tile_pool(name="ps_small", bufs=2, space="PSUM") as ps_small, \
             tc.tile_pool(name="scr_small", bufs=2) as scr_small:
            for m in range(4 if stage >= 1 else 0):
                pu = ps_small.tile([128, 128], F32, tag="pus")
                for k in range(8):
                    nc.tensor.matmul(pu[:], UWT[k][:, m * 128:(m + 1) * 128],
                                     hT[k][:], start=(k == 0), stop=(k == 7))
                u32 = scr_small.tile([128, 128], F32, tag="u32")
                nc.vector.tensor_scalar_add(u32[:], pu[:], Ub[m][:])
                nc.vector.tensor_copy(uTb[m][:], u32[:])
                nc.sync.dma_start(uT_o[m], u32[:])

                psv = ps_small.tile([128, 128], F32, tag="pus")
                for k in range(8):
                    nc.tensor.matmul(psv[:], SWT[k][:, m * 128:(m + 1) * 128],
                                     hT[k][:], start=(k == 0), stop=(k == 7))
                es = scr_small.tile([128, 128], F32, tag="es")
                nc.scalar.activation(es[:], psv[:], AF.Exp, bias=Sb[m][:])
                s32 = scr_small.tile([128, 128], F32, tag="s32")
                nc.scalar.activation(s32[:], es[:], AF.Ln, bias=1.0)
                nc.vector.tensor_copy(sTb[m][:], s32[:])
                nc.sync.dma_start(sT_o[m], s32[:])

            # ---- phase 2: zT = u + s * eps  (broadcast over the 17 variants)
            NV = N2 + 1
            for c in range(4 if stage >= 2 else 0):
                tmp = scr_small.tile([128, R], BF16, tag="ztmp")
                nc.vector.tensor_mul(
                    tmp[:].rearrange("p (v t) -> p v t", v=NV),
                    epsT[c][:].rearrange("p (v t) -> p v t", v=NV),
                    sTb[c][:, None, :].broadcast_to([128, NV, 128]))
                nc.vector.tensor_add(
                    zTb[c][:].rearrange("p (v t) -> p v t", v=NV),
                    tmp[:].rearrange("p (v t) -> p v t", v=NV),
                    uTb[c][:, None, :].broadcast_to([128, NV, 128]))

            # ---- phase 3: gathered-logit dot products
            if stage == 2:
                zc = scr_small.tile([128, 128], F32, tag="zc")
                nc.vector.tensor_copy(zc[:], zTb[0][:, 0:128])
                nc.sync.dma_start(sel1_o[:], zc[:])
            if stage >= 3:
                p1 = ps_small.tile([128, 128], F32, tag="p1", bufs=1)
                for k in range(4):
                    nc.tensor.matmul(p1[:], selT[k][:, 0:N1], zTb[k][:, 0:N1],
                                     start=(k == 0), stop=(k == 3))
                s1 = scr_small.tile([128, 128], F32, tag="s1")
                nc.vector.tensor_copy(s1[:], p1[:])
                nc.sync.dma_start(sel1_o[:], s1[:])

                p2 = ps_small.tile([16, 2048], F32, tag="p2", bufs=1)
                for k in range(4):
                    for nb in range(4):
                        nc.tensor.matmul(
                            p2[:, nb * 512:(nb + 1) * 512],
                            selT[k][:, N1:NSEL],
                            zTb[k][:, N1 + nb * 512:N1 + (nb + 1) * 512],
                            start=(k == 0), stop=(k == 3))
                s2 = scr_small.tile([16, 2048], F32, tag="s2")
                nc.vector.tensor_copy(s2[:], p2[:])
                nc.sync.dma_start(sel2_o[:], s2[:])

        # ---- phase 4: vocab-sharded logits + fused log-softmax partials
        with tc.tile_pool(name="ps_big", bufs=4, space="PSUM") as ps_big, \
             tc.tile_pool(name="scr_big", bufs=3) as scr_big:
            nrt = RT if stage >= 5 else (1 if stage == 4 else 0)
            # 1024-col PSUM tiles (2 banks), 4 in flight for deep MM/exp overlap
            VQ = (1024, 1024, 1024, 928)
            for rt in range(nrt):
                vbase = 0 if rt == 0 else VSH   # z1 rows -> fW, z2 rows -> gW
                for vq in range(4):
                    w0 = vbase + vq * 1024
                    wid = VQ[vq]
                    ps = ps_big.tile([128, 1024], F32, tag="ps")
                    for k in range(4):
                        for s0 in range(0, wid, 512):
                            w = min(512, wid - s0)
                            nc.tensor.matmul(
                                ps[:, s0:s0 + w],
                                zTb[k][:, rt * 128:(rt + 1) * 128],
                                fgWT[k][:, w0 + s0:w0 + s0 + w],
                                start=(k == 0), stop=False)
                    for s0 in range(0, wid, 512):
                        w = min(512, wid - s0)
                        nc.tensor.matmul(ps[:, s0:s0 + w], ones1[:],
                                         fgb[:, w0 + s0:w0 + s0 + w],
                                         start=False, stop=True)
                    # |logits| <= ~10 here, so exp cannot overflow fp32:
                    # skip max-stabilization, fuse exp + row-sum in one ACT op.
                    col = rt * 4 + vq
                    ex = scr_big.tile([128, 1024], BF16, tag="ex")
                    nc.scalar.activation(ex[:, :wid], ps[:, :wid], AF.Exp,
                                         accum_out=stats[:, col:col + 1])
            if nrt:
                nc.sync.dma_start(stats_o[:, :nrt * 4], stats[:, :nrt * 4])

    nc.compile()
    return nc


# ---------------------------------------------------------------- entry point
def _host_prep(inputs):
    gi = lambda n: np.asarray(inputs[n])
    words_l1 = gi("words_l1").astype(np.int64)
    words_l2 = gi("words_l2").astype(np.int64)
    emb = gi("emb").astype(np.float32)
    fW = gi("fW").astype(np.float32)
    fb = gi("fb").astype(np.float32)
    gW = gi("gW").astype(np.float32)
    gb = gi("gb").astype(np.float32)

    # host: embedding gather + sequential LSTM scans + PRNG noise
    x = emb[words_l1]
    hf = _lstm_scan_np(x, gi("Wih_f").astype(np.float32),
                       gi("Whh_f").astype(np.float32),
                       gi("bih_f").astype(np.float32),
                       gi("bhh_f").astype(np.float32))
    hb = _lstm_scan_np(x[::-1], gi("Wih_b").astype(np.float32),
                       gi("Whh_b").astype(np.float32),
                       gi("bih_b").astype(np.float32),
                       gi("bhh_b").astype(np.float32))[::-1]
    hcat = np.concatenate([hf, hb], axis=1)          # [N1, 2H]
    e1, e2 = _jax_noise()

    # per-core device inputs
    hT8 = np.ascontiguousarray(hcat.T).reshape(8, 128, 128).astype(BF)
    UW = gi("UW").astype(np.float32)
    SW = gi("SW").astype(np.float32)
    UWT = np.ascontiguousarray(UW.T).reshape(8, 128, 512).astype(BF)
    SWT = np.ascontiguousarray(SW.T).reshape(8, 128, 512).astype(BF)
    Ubc = gi("Ub").astype(np.float32).reshape(4, 128, 1)
    Sbc = gi("Sb").astype(np.float32).reshape(4, 128, 1)
    eps_all = np.concatenate([e1[None], e2], axis=0)          # [17, N1, H]
    epsT = np.ascontiguousarray(eps_all.transpose(2, 0, 1).reshape(H, R)
                                .reshape(4, 128, R)).astype(BF)
    selw = np.concatenate([fW[words_l1], gW[words_l2]], axis=0)  # [144, H]
    selT = np.ascontiguousarray(selw.T).reshape(4, 128, NSEL).astype(BF)

    fWT = np.ascontiguousarray(fW.T)   # [H, V]
    gWT = np.ascontiguousarray(gW.T)

    shared = {"hT": hT8, "UWT": UWT, "SWT": SWT, "Ub": Ubc, "Sb": Sbc,
              "epsT": epsT, "selT": selT}
    in_maps = []
    for c in range(NCORES):
        sl = slice(c * VSH, (c + 1) * VSH)
        fgWT = np.concatenate([fWT[:, sl], gWT[:, sl]], axis=1)   # [H, 8000]
        fgWT = np.ascontiguousarray(fgWT).reshape(4, 128, 2 * VSH).astype(BF)
        fgb1 = np.concatenate([fb[sl], gb[sl]]).reshape(1, 2 * VSH).astype(BF)
        in_maps.append({**shared, "fgWT": fgWT, "fgb": fgb1})
    return in_maps, {"fb": fb, "gb": gb, "words_l1": words_l1,
                     "words_l2": words_l2}


def _combine(results, aux):
    fb, gb = aux["fb"], aux["gb"]
    words_l1, words_l2 = aux["words_l1"], aux["words_l2"]
    r0 = results[0]

    u = r0["uT"].reshape(H, N1).T.astype(np.float64)
    s = r0["sT"].reshape(H, N1).T.astype(np.float64)
    kl = 0.5 * (np.sum(s * s) + np.sum(u * u) - u.size - 2.0 * np.sum(np.log(s)))

    se = np.stack([results[c]["stats"].reshape(128, RT, 4)
                   for c in range(len(results))])       # [8, 128, RT, 4]
    S = se.astype(np.float64).sum(axis=(0, 3))          # [128, RT]
    lse = np.log(S).T.reshape(R)                        # row r = v*128 + t

    l1 = np.diag(r0["sel1"]).astype(np.float64) + fb[words_l1]
    term1 = np.sum(l1 - lse[:N1])
    j = np.arange(N2)
    l2 = r0["sel2"][j[:, None], j[:, None] * 128 + np.arange(N1)[None, :]]
    l2 = l2.astype(np.float64) + gb[words_l2][:, None]
    term2 = np.sum(l2 - lse[N1:].reshape(N2, N1)) / N2

    return np.asarray(-kl + term1 + term2, dtype=np.float32)


def kernel(**inputs):
    in_maps, aux = _host_prep(inputs)
    if "prog" not in _prog_cache:
        _prog_cache["prog"] = _build_program()
    nc = _prog_cache["prog"]

    res = run_bass_kernel_spmd(nc, in_maps, list(range(NCORES)))
    global last_result
    last_result = res
    return _combine(res.results, aux)


# revision 30
# speedup vs baseline: 1.4966x; 1.4966x over previous
"""Trainium2 kernel for nn_AlignedEmbeddings (bidirectional-LSTM VAE-style loss).

Strategy (8 NeuronCores, SPMD, vocab-parallel per the sharding hint):
- Host: embedding lookup + the 0.5 GFLOP *sequential* LSTM scan (256 strictly
  serial [512]x[512,2048] matvecs — latency-bound, unsuited to the 128x128 PE
  array) + the jax PRNG noise (must bit-match jax.random).
- Device (per core, ~9 GFLOP each = 99.3% of model FLOPs): U/S projections,
  softplus, z = u + s*eps formation, the gathered-logit dot products, and the
  dominant [2176,512] @ [512,8000] vocab-sharded logit matmul (fp8 DoubleRow,
  2x PE throughput) with DVE bias-add and fused exp + row-sum (ACT accum_out)
  log-softmax partials. fW/gW vocab dim sharded 8 ways; LSTM/U/S params
  replicated. Logits are O(+-8) so exp cannot overflow fp32 and the usual
  max-stabilization pass is skipped entirely.
- Host: sums per-shard sumexp into global log-softmax normalizers, extracts
  the gathered logits, and assembles -kl + term1 + term2.
"""

import os
import sys

for _p in ("/opt/trn_rl_repo", "/opt/pypackages"):
    if os.path.isdir(_p) and _p not in sys.path:
        sys.path.append(_p)

import numpy as np
import ml_dtypes

import concourse.mybir as mybir
import concourse.tile as tile
from concourse import bacc
from concourse.bass_utils import run_bass_kernel_spmd

F32 = mybir.dt.float32
BF16 = mybir.dt.bfloat16
FP8 = mybir.dt.float8e4
AF = mybir.ActivationFunctionType
ALU = mybir.AluOpType
PM = mybir.MatmulPerfMode
BF = ml_dtypes.bfloat16
E4 = ml_dtypes.float8_e4m3

N1, N2, E, H, V = 128, 16, 256, 512, 32000
NCORES = 8
VSH = V // NCORES            # 4000 vocab columns per core
NSEL = N1 + N2               # 144 gathered weight rows
R = N1 * (N2 + 1)            # 2176 logit rows (z1 tokens + z2 samples)
RT = R // 128                # 17 row tiles
VQ = (1024, 1024, 1024, 928)  # per-row-tile vocab quarters (bank aligned)

_prog_cache = {}
last_result = None


# ---------------------------------------------------------------- host math
def _lstm_scan_np(x, Wih, Whh, bih, bhh):
    T = x.shape[0]
    WihT = np.ascontiguousarray(Wih.T)
    WhhT = np.ascontiguousarray(Whh.T)
    xg = x @ WihT + (bih + bhh)
    Hh = Whh.shape[1]
    h = np.zeros(Hh, np.float32)
    c = np.zeros(Hh, np.float32)
    hs = np.empty((T, Hh), np.float32)
    def sig(v):
        return 1.0 / (1.0 + np.exp(-v))
    for t in range(T):
        g = xg[t] + h @ WhhT
        i, f, gg, o = g[:Hh], g[Hh:2 * Hh], g[2 * Hh:3 * Hh], g[3 * Hh:]
        c = sig(f) * c + sig(i) * np.tanh(gg)
        h = sig(o) * np.tanh(c)
        hs[t] = h
    return hs


def _jax_noise():
    """eps1 [N1,H], eps2 [N2,N1,H] from jax.random key(42), computed on CPU."""
    try:
        import jax
        cpus = jax.devices("cpu")
        with jax.default_device(cpus[0]):
            k1, k2 = jax.random.split(jax.random.key(42))
            e1 = np.asarray(jax.random.normal(k1, (N1, H), np.float32))
            e2 = np.asarray(jax.random.normal(k2, (N2, N1, H), np.float32))
        return e1, e2
    except Exception:
        pass
    # Fallback: clean-env subprocess pinned to the CPU backend.
    import subprocess, tempfile
    code = (
        "import numpy as np, jax\n"
        "k1, k2 = jax.random.split(jax.random.key(42))\n"
        f"e1 = np.asarray(jax.random.normal(k1, ({N1}, {H}), np.float32))\n"
        f"e2 = np.asarray(jax.random.normal(k2, ({N2}, {N1}, {H}), np.float32))\n"
        "import sys; np.savez(sys.argv[1], e1=e1, e2=e2)\n"
    )
    env = dict(os.environ)
    env["JAX_PLATFORMS"] = "cpu"
    with tempfile.TemporaryDirectory() as td:
        out = os.path.join(td, "noise.npz")
        subprocess.run([sys.executable, "-c", code, out], check=True, env=env)
        z = np.load(out)
        return z["e1"], z["e2"]


# ---------------------------------------------------------------- device prog
def _build_program(stage=5):
    nc = bacc.Bacc("TRN2", target_bir_lowering=False, debug=False,
                   num_devices=NCORES)

    def din(name, shape, dt):
        return nc.dram_tensor(name, shape, dt, kind="ExternalInput").ap()

    def dout(name, shape, dt):
        return nc.dram_tensor(name, shape, dt, kind="ExternalOutput").ap()

    husw_d = din("husw", [8, 128, 1152], BF16)   # hT | UWT | SWT per k-chunk
    Ub_d = din("Ub", [4, 128, 1], F32)
    Sb_d = din("Sb", [4, 128, 1], F32)
    eps_d = din("epsT", [4, 128, R], BF16)
    sel_d = din("selT", [4, 128, NSEL], FP8)
    fgW_d = din("fgWT", [2, 128, 2, 2 * VSH], FP8)   # k-pair interleaved
    fgb_d = din("fgb", [128, 2 * VSH], BF16)          # replicated bias rows

    uT_o = dout("uT", [4, 128, 128], F32)
    sT_o = dout("sT", [4, 128, 128], F32)
    sel1_o = dout("sel1", [128, 128], F32)
    sel2_o = dout("sel2", [16, 2048], F32)
    stats_o = dout("stats", [128, RT * 4], F32)

    from contextlib import ExitStack
    with tile.TileContext(nc) as tc, ExitStack() as stack:
        con = stack.enter_context(tc.tile_pool(name="con", bufs=1))
        husw = [con.tile([128, 1152], BF16, tag=f"husw{k}", name=f"husw{k}")
                for k in range(8)]
        hT = [t[:, 0:128] for t in husw]
        UWT = [t[:, 128:640] for t in husw]
        SWT = [t[:, 640:1152] for t in husw]
        Ub = [con.tile([128, 1], F32, tag=f"Ub{m}", name=f"Ub{m}") for m in range(4)]
        Sb = [con.tile([128, 1], F32, tag=f"Sb{m}", name=f"Sb{m}") for m in range(4)]
        epsT = [con.tile([128, R], BF16, tag=f"eps{c}", name=f"eps{c}") for c in range(4)]
        selT = [con.tile([128, NSEL], FP8, tag=f"sel{c}", name=f"selc{c}") for c in range(4)]
        fgWp = [con.tile([128, 2, 2 * VSH], FP8, tag=f"fgW{p}", name=f"fgW{p}")
                for p in range(2)]
        fgb = con.tile([128, 2 * VSH], BF16, tag="fgb")

        # All DMAs on the two hardware-DGE queues (SP + ACT), urgent first.
        eng = [nc.sync, nc.scalar]
        for k in range(8):
            eng[k % 2].dma_start(husw[k][:], husw_d[k])
        for m in range(4):
            nc.sync.dma_start(Ub[m][:], Ub_d[m])
            nc.scalar.dma_start(Sb[m][:], Sb_d[m])
        for c in range(4):
            eng[c % 2].dma_start(epsT[c][:], eps_d[c])
            eng[(c + 1) % 2].dma_start(selT[c][:], sel_d[c])
        # f-half first (row-tile 0), then g-half; bias rows alongside
        for half in range(2):
            for p in range(2):
                eng[p % 2].dma_start(
                    fgWp[p][:, :, half * VSH:(half + 1) * VSH],
                    fgW_d[p][:, :, half * VSH:(half + 1) * VSH])
            eng[half % 2].dma_start(fgb[:, half * VSH:(half + 1) * VSH],
                                    fgb_d[:, half * VSH:(half + 1) * VSH])

        work = stack.enter_context(tc.tile_pool(name="work", bufs=1))
        uTb = [work.tile([128, 128], BF16, tag=f"uTb{m}", name=f"uTb{m}") for m in range(4)]
        sTb = [work.tile([128, 128], BF16, tag=f"sTb{m}", name=f"sTb{m}") for m in range(4)]
        zTp = [work.tile([128, 2, R], FP8, tag=f"zTp{p}", name=f"zTp{p}") for p in range(2)]
        stats = work.tile([128, RT * 4], F32, tag="stats")

        def zch(c):          # z chunk c as [128, cols...] AP
            return zTp[c // 2][:, c % 2]

        # ---- phase 1: uT, sT  (uT = UW @ hT + Ub ; sT = softplus(SW @ hT + Sb))
        with tc.tile_pool(name="ps_small", bufs=2, space="PSUM") as ps_small, \
             tc.tile_pool(name="scr_small", bufs=2) as scr_small:
            for m in range(4 if stage >= 1 else 0):
                pu = ps_small.tile([128, 128], F32, tag="pus")
                for k in range(8):
                    nc.tensor.matmul(pu[:], UWT[k][:, m * 128:(m + 1) * 128],
                                     hT[k][:], start=(k == 0), stop=(k == 7))
                u32 = scr_small.tile([128, 128], F32, tag="u32")
                nc.vector.tensor_scalar_add(u32[:], pu[:], Ub[m][:])
                nc.vector.tensor_copy(uTb[m][:], u32[:])
                nc.sync.dma_start(uT_o[m], u32[:])

                psv = ps_small.tile([128, 128], F32, tag="pus")
                for k in range(8):
                    nc.tensor.matmul(psv[:], SWT[k][:, m * 128:(m + 1) * 128],
                                     hT[k][:], start=(k == 0), stop=(k == 7))
                es = scr_small.tile([128, 128], F32, tag="es")
                nc.scalar.activation(es[:], psv[:], AF.Exp, bias=Sb[m][:])
                s32 = scr_small.tile([128, 128], F32, tag="s32")
                nc.scalar.activation(s32[:], es[:], AF.Ln, bias=1.0)
                nc.vector.tensor_copy(sTb[m][:], s32[:])
                nc.sync.dma_start(sT_o[m], s32[:])

            # ---- phase 2: zT = u + s * eps  (broadcast over the 17 variants)
            NV = N2 + 1
            for c in range(4 if stage >= 2 else 0):
                tmp = scr_small.tile([128, R], BF16, tag="ztmp")
                nc.vector.tensor_mul(
                    tmp[:].rearrange("p (v t) -> p v t", v=NV),
                    epsT[c][:].rearrange("p (v t) -> p v t", v=NV),
                    sTb[c][:, None, :].broadcast_to([128, NV, 128]))
                nc.vector.tensor_add(
                    zch(c).rearrange("p (v t) -> p v t", v=NV),
                    tmp[:].rearrange("p (v t) -> p v t", v=NV),
                    uTb[c][:, None, :].broadcast_to([128, NV, 128]))

            # ---- phase 3: gathered-logit dot products
            if stage == 2:
                zc = scr_small.tile([128, 128], F32, tag="zc")
                nc.vector.tensor_copy(zc[:], zch(0)[:, 0:128])
                nc.sync.dma_start(sel1_o[:], zc[:])
            if stage >= 3:
                p1 = ps_small.tile([128, 128], F32, tag="p1", bufs=1)
                for k in range(4):
                    nc.tensor.matmul(p1[:], selT[k][:, 0:N1], zch(k)[:, 0:N1],
                                     start=(k == 0), stop=(k == 3))
                s1 = scr_small.tile([128, 128], F32, tag="s1")
                nc.vector.tensor_copy(s1[:], p1[:])
                nc.sync.dma_start(sel1_o[:], s1[:])

                p2 = ps_small.tile([16, 2048], F32, tag="p2", bufs=1)
                for k in range(4):
                    for nb in range(4):
                        nc.tensor.matmul(
                            p2[:, nb * 512:(nb + 1) * 512],
                            selT[k][:, N1:NSEL],
                            zch(k)[:, N1 + nb * 512:N1 + (nb + 1) * 512],
                            start=(k == 0), stop=(k == 3))
                s2 = scr_small.tile([16, 2048], F32, tag="s2")
                nc.vector.tensor_copy(s2[:], p2[:])
                nc.sync.dma_start(sel2_o[:], s2[:])

        # ---- phase 4: vocab-sharded logits + fused log-softmax partials
        with tc.tile_pool(name="ps_big", bufs=4, space="PSUM") as ps_big, \
             tc.tile_pool(name="scr_big", bufs=3) as scr_big:
            nrt = RT if stage >= 5 else (1 if stage == 4 else 0)
            for rt in range(nrt):
                vbase = 0 if rt == 0 else VSH   # z1 rows -> fW, z2 rows -> gW
                for vq in range(4):
                    w0 = vbase + vq * 1024
                    wid = VQ[vq]
                    ps = ps_big.tile([128, 1024], F32, tag="ps")
                    for p in range(2):
                        for s0 in range(0, wid, 512):
                            w = min(512, wid - s0)
                            nc.tensor.matmul(
                                ps[:, s0:s0 + w],
                                zTp[p][:, :, rt * 128:(rt + 1) * 128],
                                fgWp[p][:, :, w0 + s0:w0 + s0 + w],
                                start=(p == 0), stop=(p == 1),
                                perf_mode=PM.DoubleRow)
                    # bias add on DVE (psum f32 + bf16 row-replica -> bf16)
                    badd = scr_big.tile([128, 1024], BF16, tag="badd")
                    nc.vector.tensor_add(badd[:, :wid], ps[:, :wid],
                                         fgb[:, w0:w0 + wid])
                    # |logits| <= ~10 here, so exp cannot overflow fp32:
                    # skip max-stabilization, fuse exp + row-sum in one ACT op.
                    col = rt * 4 + vq
                    ex = scr_big.tile([128, 1024], BF16, tag="ex")
                    nc.scalar.activation(ex[:, :wid], badd[:, :wid], AF.Exp,
                                         accum_out=stats[:, col:col + 1])
            if nrt:
                nc.sync.dma_start(stats_o[:, :nrt * 4], stats[:, :nrt * 4])

    nc.compile()
    return nc


# ---------------------------------------------------------------- entry point
def _host_prep(inputs):
    gi = lambda n: np.asarray(inputs[n])
    words_l1 = gi("words_l1").astype(np.int64)
    words_l2 = gi("words_l2").astype(np.int64)
    emb = gi("emb").astype(np.float32)
    fW = gi("fW").astype(np.float32)
    fb = gi("fb").astype(np.float32)
    gW = gi("gW").astype(np.float32)
    gb = gi("gb").astype(np.float32)

    # host: embedding gather + sequential LSTM scans + PRNG noise
    x = emb[words_l1]
    hf = _lstm_scan_np(x, gi("Wih_f").astype(np.float32),
                       gi("Whh_f").astype(np.float32),
                       gi("bih_f").astype(np.float32),
                       gi("bhh_f").astype(np.float32))
    hb = _lstm_scan_np(x[::-1], gi("Wih_b").astype(np.float32),
                       gi("Whh_b").astype(np.float32),
                       gi("bih_b").astype(np.float32),
                       gi("bhh_b").astype(np.float32))[::-1]
    hcat = np.concatenate([hf, hb], axis=1)          # [N1, 2H]
    e1, e2 = _jax_noise()

    hT8 = np.ascontiguousarray(hcat.T).reshape(8, 128, 128)
    UW = gi("UW").astype(np.float32)
    SW = gi("SW").astype(np.float32)
    UWT = np.ascontiguousarray(UW.T).reshape(8, 128, 512)
    SWT = np.ascontiguousarray(SW.T).reshape(8, 128, 512)
    husw = np.concatenate([hT8, UWT, SWT], axis=2).astype(BF)  # [8,128,1152]
    Ubc = gi("Ub").astype(np.float32).reshape(4, 128, 1)
    Sbc = gi("Sb").astype(np.float32).reshape(4, 128, 1)
    eps_all = np.concatenate([e1[None], e2], axis=0)          # [17, N1, H]
    epsT = np.ascontiguousarray(eps_all.transpose(2, 0, 1).reshape(H, R)
                                .reshape(4, 128, R)).astype(BF)
    selw = np.concatenate([fW[words_l1], gW[words_l2]], axis=0)  # [144, H]
    selT = np.ascontiguousarray(selw.T).reshape(4, 128, NSEL).astype(E4)

    fWT = np.ascontiguousarray(fW.T)   # [H, V]
    gWT = np.ascontiguousarray(gW.T)

    shared = {"husw": husw, "Ub": Ubc, "Sb": Sbc, "epsT": epsT, "selT": selT}
    in_maps = []
    for c in range(NCORES):
        sl = slice(c * VSH, (c + 1) * VSH)
        fgWT = np.concatenate([fWT[:, sl], gWT[:, sl]], axis=1)   # [H, 8000]
        # k-pair interleave: [2 pairs, 128 Ki, 2 planes, 8000]
        fgWT = np.ascontiguousarray(
            fgWT.reshape(2, 2, 128, 2 * VSH).transpose(0, 2, 1, 3)).astype(E4)
        fgb1 = np.concatenate([fb[sl], gb[sl]]).astype(np.float32)
        fgbr = np.ascontiguousarray(
            np.broadcast_to(fgb1, (128, 2 * VSH))).astype(BF)
        in_maps.append({**shared, "fgWT": fgWT, "fgb": fgbr})
    return in_maps, {"fb": fb, "gb": gb, "words_l1": words_l1,
                     "words_l2": words_l2}


def _combine(results, aux):
    fb, gb = aux["fb"], aux["gb"]
    words_l1, words_l2 = aux["words_l1"], aux["words_l2"]
    r0 = results[0]

    u = r0["uT"].reshape(H, N1).T.astype(np.float64)
    s = r0["sT"].reshape(H, N1).T.astype(np.float64)
    kl = 0.5 * (np.sum(s * s) + np.sum(u * u) - u.size - 2.0 * np.sum(np.log(s)))

    se = np.stack([results[c]["stats"].reshape(128, RT, 4)
                   for c in range(len(results))])       # [8, 128, RT, 4]
    S = se.astype(np.float64).sum(axis=(0, 3))          # [128, RT]
    lse = np.log(S).T.reshape(R)                        # row r = v*128 + t

    l1 = np.diag(r0["sel1"]).astype(np.float64) + fb[words_l1]
    term1 = np.sum(l1 - lse[:N1])
    j = np.arange(N2)
    l2 = r0["sel2"][j[:, None], j[:, None] * 128 + np.arange(N1)[None, :]]
    l2 = l2.astype(np.float64) + gb[words_l2][:, None]
    term2 = np.sum(l2 - lse[N1:].reshape(N2, N1)) / N2

    return np.asarray(-kl + term1 + term2, dtype=np.float32)


def kernel(**inputs):
    in_maps, aux = _host_prep(inputs)
    if "prog" not in _prog_cache:
        _prog_cache["prog"] = _build_program()
    nc = _prog_cache["prog"]

    res = run_bass_kernel_spmd(nc, in_maps, list(range(NCORES)))
    global last_result
    last_result = res
    return _combine(res.results, aux)


# revision 32
# speedup vs baseline: 1.5997x; 1.0689x over previous
"""Trainium2 kernel for nn_AlignedEmbeddings (bidirectional-LSTM VAE-style loss).

Strategy (8 NeuronCores, SPMD, vocab-parallel per the sharding hint):
- Host: embedding lookup + the 0.5 GFLOP *sequential* LSTM scan (256 strictly
  serial [512]x[512,2048] matvecs — latency-bound, unsuited to the 128x128 PE
  array) + the jax PRNG noise (must bit-match jax.random).
- Device (per core, ~9 GFLOP each = 99.3% of model FLOPs): U/S projections,
  softplus, z = u + s*eps formation, the gathered-logit dot products, and the
  dominant [2176,512] @ [512,8000] vocab-sharded logit matmul (fp8 DoubleRow,
  2x PE throughput) with DVE bias-add and fused exp + row-sum (ACT accum_out)
  log-softmax partials. fW/gW vocab dim sharded 8 ways; LSTM/U/S params
  replicated. Logits are O(+-8) so exp cannot overflow fp32 and the usual
  max-stabilization pass is skipped entirely.
- Host: sums per-shard sumexp into global log-softmax normalizers, extracts
  the gathered logits, and assembles -kl + term1 + term2.
"""

import os
import sys

for _p in ("/opt/trn_rl_repo", "/opt/pypackages"):
    if os.path.isdir(_p) and _p not in sys.path:
        sys.path.append(_p)

import numpy as np
import ml_dtypes

import concourse.mybir as mybir
import concourse.tile as tile
from concourse import bacc
from concourse.bass_utils import run_bass_kernel_spmd

F32 = mybir.dt.float32
BF16 = mybir.dt.bfloat16
FP8 = mybir.dt.float8e4
AF = mybir.ActivationFunctionType
ALU = mybir.AluOpType
PM = mybir.MatmulPerfMode
BF = ml_dtypes.bfloat16
E4 = ml_dtypes.float8_e4m3

N1, N2, E, H, V = 128, 16, 256, 512, 32000
NCORES = 8
VSH = V // NCORES            # 4000 vocab columns per core
NSEL = N1 + N2               # 144 gathered weight rows
R = N1 * (N2 + 1)            # 2176 logit rows (z1 tokens + z2 samples)
RT = R // 128                # 17 row tiles
VQ = (1024, 1024, 1024, 928)  # per-row-tile vocab quarters (bank aligned)

_prog_cache = {}
last_result = None


# ---------------------------------------------------------------- host math
def _lstm_scan_np(x, Wih, Whh, bih, bhh):
    T = x.shape[0]
    WihT = np.ascontiguousarray(Wih.T)
    WhhT = np.ascontiguousarray(Whh.T)
    xg = x @ WihT + (bih + bhh)
    Hh = Whh.shape[1]
    h = np.zeros(Hh, np.float32)
    c = np.zeros(Hh, np.float32)
    hs = np.empty((T, Hh), np.float32)
    def sig(v):
        return 1.0 / (1.0 + np.exp(-v))
    for t in range(T):
        g = xg[t] + h @ WhhT
        i, f, gg, o = g[:Hh], g[Hh:2 * Hh], g[2 * Hh:3 * Hh], g[3 * Hh:]
        c = sig(f) * c + sig(i) * np.tanh(gg)
        h = sig(o) * np.tanh(c)
        hs[t] = h
    return hs


def _jax_noise():
    """eps1 [N1,H], eps2 [N2,N1,H] from jax.random key(42), computed on CPU."""
    try:
        import jax
        cpus = jax.devices("cpu")
        with jax.default_device(cpus[0]):
            k1, k2 = jax.random.split(jax.random.key(42))
            e1 = np.asarray(jax.random.normal(k1, (N1, H), np.float32))
            e2 = np.asarray(jax.random.normal(k2, (N2, N1, H), np.float32))
        return e1, e2
    except Exception:
        pass
    # Fallback: clean-env subprocess pinned to the CPU backend.
    import subprocess, tempfile
    code = (
        "import numpy as np, jax\n"
        "k1, k2 = jax.random.split(jax.random.key(42))\n"
        f"e1 = np.asarray(jax.random.normal(k1, ({N1}, {H}), np.float32))\n"
        f"e2 = np.asarray(jax.random.normal(k2, ({N2}, {N1}, {H}), np.float32))\n"
        "import sys; np.savez(sys.argv[1], e1=e1, e2=e2)\n"
    )
    env = dict(os.environ)
    env["JAX_PLATFORMS"] = "cpu"
    with tempfile.TemporaryDirectory() as td:
        out = os.path.join(td, "noise.npz")
        subprocess.run([sys.executable, "-c", code, out], check=True, env=env)
        z = np.load(out)
        return z["e1"], z["e2"]


# ---------------------------------------------------------------- device prog
def _build_program(stage=5):
    nc = bacc.Bacc("TRN2", target_bir_lowering=False, debug=False,
                   num_devices=NCORES)

    def din(name, shape, dt):
        return nc.dram_tensor(name, shape, dt, kind="ExternalInput").ap()

    def dout(name, shape, dt):
        return nc.dram_tensor(name, shape, dt, kind="ExternalOutput").ap()

    husw_d = din("husw", [8, 128, 1152], BF16)   # hT | UWT | SWT per k-chunk
    Ub_d = din("Ub", [4, 128, 1], F32)
    Sb_d = din("Sb", [4, 128, 1], F32)
    eps_d = din("epsT", [4, 128, R], BF16)
    sel_d = din("selT", [4, 128, NSEL], FP8)
    fgW_d = din("fgWT", [2, 128, 2, 2 * VSH], FP8)   # k-pair interleaved
    fgb_d = din("fgb", [128, 2 * VSH], BF16)          # replicated bias rows

    uT_o = dout("uT", [4, 128, 128], F32)
    sT_o = dout("sT", [4, 128, 128], F32)
    sel1_o = dout("sel1", [128, 128], F32)
    sel2_o = dout("sel2", [16, 2048], F32)
    stats_o = dout("stats", [128, RT * 2], F32)

    from contextlib import ExitStack
    with tile.TileContext(nc) as tc, ExitStack() as stack:
        con = stack.enter_context(tc.tile_pool(name="con", bufs=1))
        husw = [con.tile([128, 1152], BF16, tag=f"husw{k}", name=f"husw{k}")
                for k in range(8)]
        hT = [t[:, 0:128] for t in husw]
        UWT = [t[:, 128:640] for t in husw]
        SWT = [t[:, 640:1152] for t in husw]
        Ub = [con.tile([128, 1], F32, tag=f"Ub{m}", name=f"Ub{m}") for m in range(4)]
        Sb = [con.tile([128, 1], F32, tag=f"Sb{m}", name=f"Sb{m}") for m in range(4)]
        epsT = [con.tile([128, R], BF16, tag=f"eps{c}", name=f"eps{c}") for c in range(4)]
        selT = [con.tile([128, NSEL], FP8, tag=f"sel{c}", name=f"selc{c}") for c in range(4)]
        fgWp = [con.tile([128, 2, 2 * VSH], FP8, tag=f"fgW{p}", name=f"fgW{p}")
                for p in range(2)]
        fgb = con.tile([128, 2 * VSH], BF16, tag="fgb")

        # All DMAs on the two hardware-DGE queues (SP + ACT), urgent first.
        eng = [nc.sync, nc.scalar]
        for k in range(8):
            eng[k % 2].dma_start(husw[k][:], husw_d[k])
        for m in range(4):
            nc.sync.dma_start(Ub[m][:], Ub_d[m])
            nc.scalar.dma_start(Sb[m][:], Sb_d[m])
        for c in range(4):
            eng[c % 2].dma_start(epsT[c][:], eps_d[c])
            eng[(c + 1) % 2].dma_start(selT[c][:], sel_d[c])
        # f-half first (row-tile 0), then g-half; bias rows alongside
        for half in range(2):
            for p in range(2):
                eng[p % 2].dma_start(
                    fgWp[p][:, :, half * VSH:(half + 1) * VSH],
                    fgW_d[p][:, :, half * VSH:(half + 1) * VSH])
            eng[half % 2].dma_start(fgb[:, half * VSH:(half + 1) * VSH],
                                    fgb_d[:, half * VSH:(half + 1) * VSH])

        work = stack.enter_context(tc.tile_pool(name="work", bufs=1))
        uTb = [work.tile([128, 128], BF16, tag=f"uTb{m}", name=f"uTb{m}") for m in range(4)]
        sTb = [work.tile([128, 128], BF16, tag=f"sTb{m}", name=f"sTb{m}") for m in range(4)]
        zTp = [work.tile([128, 2, R], FP8, tag=f"zTp{p}", name=f"zTp{p}") for p in range(2)]
        stats = work.tile([128, RT * 2], F32, tag="stats")

        def zch(c):          # z chunk c as [128, cols...] AP
            return zTp[c // 2][:, c % 2]

        # ---- phase 1: uT, sT  (uT = UW @ hT + Ub ; sT = softplus(SW @ hT + Sb))
        with tc.tile_pool(name="ps_small", bufs=2, space="PSUM") as ps_small, \
             tc.tile_pool(name="scr_small", bufs=2) as scr_small:
            for m in range(4 if stage >= 1 else 0):
                pu = ps_small.tile([128, 128], F32, tag="pus")
                for k in range(8):
                    nc.tensor.matmul(pu[:], UWT[k][:, m * 128:(m + 1) * 128],
                                     hT[k][:], start=(k == 0), stop=(k == 7))
                u32 = scr_small.tile([128, 128], F32, tag="u32")
                nc.vector.tensor_scalar_add(u32[:], pu[:], Ub[m][:])
                nc.vector.tensor_copy(uTb[m][:], u32[:])
                nc.sync.dma_start(uT_o[m], u32[:])

                psv = ps_small.tile([128, 128], F32, tag="pus")
                for k in range(8):
                    nc.tensor.matmul(psv[:], SWT[k][:, m * 128:(m + 1) * 128],
                                     hT[k][:], start=(k == 0), stop=(k == 7))
                es = scr_small.tile([128, 128], F32, tag="es")
                nc.scalar.activation(es[:], psv[:], AF.Exp, bias=Sb[m][:])
                s32 = scr_small.tile([128, 128], F32, tag="s32")
                nc.scalar.activation(s32[:], es[:], AF.Ln, bias=1.0)
                nc.vector.tensor_copy(sTb[m][:], s32[:])
                nc.sync.dma_start(sT_o[m], s32[:])

            # ---- phase 2: zT = u + s * eps  (broadcast over the 17 variants)
            NV = N2 + 1
            for c in range(4 if stage >= 2 else 0):
                tmp = scr_small.tile([128, R], BF16, tag="ztmp")
                nc.vector.tensor_mul(
                    tmp[:].rearrange("p (v t) -> p v t", v=NV),
                    epsT[c][:].rearrange("p (v t) -> p v t", v=NV),
                    sTb[c][:, None, :].broadcast_to([128, NV, 128]))
                nc.vector.tensor_add(
                    zch(c).rearrange("p (v t) -> p v t", v=NV),
                    tmp[:].rearrange("p (v t) -> p v t", v=NV),
                    uTb[c][:, None, :].broadcast_to([128, NV, 128]))

            # ---- phase 3: gathered-logit dot products
            if stage == 2:
                zc = scr_small.tile([128, 128], F32, tag="zc")
                nc.vector.tensor_copy(zc[:], zch(0)[:, 0:128])
                nc.sync.dma_start(sel1_o[:], zc[:])
            if stage >= 3:
                p1 = ps_small.tile([128, 128], F32, tag="p1", bufs=1)
                for k in range(4):
                    nc.tensor.matmul(p1[:], selT[k][:, 0:N1], zch(k)[:, 0:N1],
                                     start=(k == 0), stop=(k == 3))
                s1 = scr_small.tile([128, 128], F32, tag="s1")
                nc.vector.tensor_copy(s1[:], p1[:])
                nc.sync.dma_start(sel1_o[:], s1[:])

                p2 = ps_small.tile([16, 2048], F32, tag="p2", bufs=1)
                for k in range(4):
                    for nb in range(4):
                        nc.tensor.matmul(
                            p2[:, nb * 512:(nb + 1) * 512],
                            selT[k][:, N1:NSEL],
                            zch(k)[:, N1 + nb * 512:N1 + (nb + 1) * 512],
                            start=(k == 0), stop=(k == 3))
                s2 = scr_small.tile([16, 2048], F32, tag="s2")
                nc.vector.tensor_copy(s2[:], p2[:])
                nc.sync.dma_start(sel2_o[:], s2[:])

        # ---- phase 4: vocab-sharded logits + fused log-softmax partials
        with tc.tile_pool(name="ps_big", bufs=4, space="PSUM") as ps_big, \
             tc.tile_pool(name="scr_big", bufs=3) as scr_big:
            nrt = RT if stage >= 5 else (1 if stage == 4 else 0)
            for rt in range(nrt):
                vbase = 0 if rt == 0 else VSH   # z1 rows -> fW, z2 rows -> gW
                badd = None
                for vq in range(4):
                    w0 = vbase + vq * 1024
                    wid = VQ[vq]
                    ps = ps_big.tile([128, 1024], F32, tag="ps")
                    for p in range(2):
                        for s0 in range(0, wid, 512):
                            w = min(512, wid - s0)
                            nc.tensor.matmul(
                                ps[:, s0:s0 + w],
                                zTp[p][:, :, rt * 128:(rt + 1) * 128],
                                fgWp[p][:, :, w0 + s0:w0 + s0 + w],
                                start=(p == 0), stop=(p == 1),
                                perf_mode=PM.DoubleRow)
                    # bias add on DVE (psum f32 + bf16 row-replica -> bf16),
                    # pairs of quarters share one scratch for a wide exp op
                    half = vq // 2
                    if vq % 2 == 0:
                        badd = scr_big.tile([128, 2048], BF16, tag="badd")
                    nc.vector.tensor_add(
                        badd[:, vq % 2 * 1024:vq % 2 * 1024 + wid],
                        ps[:, :wid], fgb[:, w0:w0 + wid])
                    if vq % 2 == 1:
                        # |logits| <= ~10, exp cannot overflow fp32: skip
                        # max-stabilization, fuse exp + row-sum in one ACT op.
                        hw_ = 1024 + wid
                        col = rt * 2 + half
                        ex = scr_big.tile([128, 2048], BF16, tag="ex")
                        nc.scalar.activation(ex[:, :hw_], badd[:, :hw_],
                                             AF.Exp,
                                             accum_out=stats[:, col:col + 1])
            if nrt:
                nc.sync.dma_start(stats_o[:, :nrt * 2], stats[:, :nrt * 2])

    nc.compile()
    return nc


# ---------------------------------------------------------------- entry point
def _host_prep(inputs):
    gi = lambda n: np.asarray(inputs[n])
    words_l1 = gi("words_l1").astype(np.int64)
    words_l2 = gi("words_l2").astype(np.int64)
    emb = gi("emb").astype(np.float32)
    fW = gi("fW").astype(np.float32)
    fb = gi("fb").astype(np.float32)
    gW = gi("gW").astype(np.float32)
    gb = gi("gb").astype(np.float32)

    # host: embedding gather + sequential LSTM scans + PRNG noise
    x = emb[words_l1]
    hf = _lstm_scan_np(x, gi("Wih_f").astype(np.float32),
                       gi("Whh_f").astype(np.float32),
                       gi("bih_f").astype(np.float32),
                       gi("bhh_f").astype(np.float32))
    hb = _lstm_scan_np(x[::-1], gi("Wih_b").astype(np.float32),
                       gi("Whh_b").astype(np.float32),
                       gi("bih_b").astype(np.float32),
                       gi("bhh_b").astype(np.float32))[::-1]
    hcat = np.concatenate([hf, hb], axis=1)          # [N1, 2H]
    e1, e2 = _jax_noise()

    hT8 = np.ascontiguousarray(hcat.T).reshape(8, 128, 128)
    UW = gi("UW").astype(np.float32)
    SW = gi("SW").astype(np.float32)
    UWT = np.ascontiguousarray(UW.T).reshape(8, 128, 512)
    SWT = np.ascontiguousarray(SW.T).reshape(8, 128, 512)
    husw = np.concatenate([hT8, UWT, SWT], axis=2).astype(BF)  # [8,128,1152]
    Ubc = gi("Ub").astype(np.float32).reshape(4, 128, 1)
    Sbc = gi("Sb").astype(np.float32).reshape(4, 128, 1)
    eps_all = np.concatenate([e1[None], e2], axis=0)          # [17, N1, H]
    epsT = np.ascontiguousarray(eps_all.transpose(2, 0, 1).reshape(H, R)
                                .reshape(4, 128, R)).astype(BF)
    selw = np.concatenate([fW[words_l1], gW[words_l2]], axis=0)  # [144, H]
    selT = np.ascontiguousarray(selw.T).reshape(4, 128, NSEL).astype(E4)

    fWT = np.ascontiguousarray(fW.T)   # [H, V]
    gWT = np.ascontiguousarray(gW.T)

    shared = {"husw": husw, "Ub": Ubc, "Sb": Sbc, "epsT": epsT, "selT": selT}
    in_maps = []
    for c in range(NCORES):
        sl = slice(c * VSH, (c + 1) * VSH)
        fgWT = np.concatenate([fWT[:, sl], gWT[:, sl]], axis=1)   # [H, 8000]
        # k-pair interleave: [2 pairs, 128 Ki, 2 planes, 8000]
        fgWT = np.ascontiguousarray(
            fgWT.reshape(2, 2, 128, 2 * VSH).transpose(0, 2, 1, 3)).astype(E4)
        fgb1 = np.concatenate([fb[sl], gb[sl]]).astype(np.float32)
        fgbr = np.ascontiguousarray(
            np.broadcast_to(fgb1, (128, 2 * VSH))).astype(BF)
        in_maps.append({**shared, "fgWT": fgWT, "fgb": fgbr})
    return in_maps, {"fb": fb, "gb": gb, "words_l1": words_l1,
                     "words_l2": words_l2}


def _combine(results, aux):
    fb, gb = aux["fb"], aux["gb"]
    words_l1, words_l2 = aux["words_l1"], aux["words_l2"]
    r0 = results[0]

    u = r0["uT"].reshape(H, N1).T.astype(np.float64)
    s = r0["sT"].reshape(H, N1).T.astype(np.float64)
    kl = 0.5 * (np.sum(s * s) + np.sum(u * u) - u.size - 2.0 * np.sum(np.log(s)))

    se = np.stack([results[c]["stats"].reshape(128, RT, 2)
                   for c in range(len(results))])       # [8, 128, RT, 2]
    S = se.astype(np.float64).sum(axis=(0, 3))          # [128, RT]
    lse = np.log(S).T.reshape(R)                        # row r = v*128 + t

    l1 = np.diag(r0["sel1"]).astype(np.float64) + fb[words_l1]
    term1 = np.sum(l1 - lse[:N1])
    j = np.arange(N2)
    l2 = r0["sel2"][j[:, None], j[:, None] * 128 + np.arange(N1)[None, :]]
    l2 = l2.astype(np.float64) + gb[words_l2][:, None]
    term2 = np.sum(l2 - lse[N1:].reshape(N2, N1)) / N2

    return np.asarray(-kl + term1 + term2, dtype=np.float32)


def kernel(**inputs):
    in_maps, aux = _host_prep(inputs)
    if "prog" not in _prog_cache:
        _prog_cache["prog"] = _build_program()
    nc = _prog_cache["prog"]

    res = run_bass_kernel_spmd(nc, in_maps, list(range(NCORES)))
    global last_result
    last_result = res
    return _combine(res.results, aux)


# revision 37
# speedup vs baseline: 1.6194x; 1.0123x over previous
"""Trainium2 kernel for nn_AlignedEmbeddings (bidirectional-LSTM VAE-style loss).

Strategy (8 NeuronCores, SPMD, vocab-parallel per the sharding hint):
- Host: embedding lookup + the 0.5 GFLOP *sequential* LSTM scan (256 strictly
  serial [512]x[512,2048] matvecs — latency-bound, unsuited to the 128x128 PE
  array) + the jax PRNG noise (must bit-match jax.random).
- Device (per core, ~9 GFLOP each = 99.3% of model FLOPs): U/S projections,
  softplus, z = u + s*eps formation, the gathered-logit dot products, and the
  dominant [2176,512] @ [512,8000] vocab-sharded logit matmul (fp8 DoubleRow,
  2x PE throughput) with DVE bias-add and fused exp + row-sum (ACT accum_out)
  log-softmax partials. fW/gW vocab dim sharded 8 ways; LSTM/U/S params
  replicated. Logits are O(+-8) so exp cannot overflow fp32 and the usual
  max-stabilization pass is skipped entirely.
- Host: sums per-shard sumexp into global log-softmax normalizers, extracts
  the gathered logits, and assembles -kl + term1 + term2.
"""

import os
import sys

for _p in ("/opt/trn_rl_repo", "/opt/pypackages"):
    if os.path.isdir(_p) and _p not in sys.path:
        sys.path.append(_p)

import numpy as np
import ml_dtypes

import concourse.mybir as mybir
import concourse.tile as tile
from concourse import bacc
from concourse.bass_utils import run_bass_kernel_spmd

F32 = mybir.dt.float32
BF16 = mybir.dt.bfloat16
FP8 = mybir.dt.float8e4
AF = mybir.ActivationFunctionType
ALU = mybir.AluOpType
PM = mybir.MatmulPerfMode
BF = ml_dtypes.bfloat16
E4 = ml_dtypes.float8_e4m3

N1, N2, E, H, V = 128, 16, 256, 512, 32000
NCORES = 8
VSH = V // NCORES            # 4000 vocab columns per core
NSEL = N1 + N2               # 144 gathered weight rows
R = N1 * (N2 + 1)            # 2176 logit rows (z1 tokens + z2 samples)
RT = R // 128                # 17 row tiles
VQ = (1024, 1024, 1024, 928)  # per-row-tile vocab quarters (bank aligned)

_prog_cache = {}
last_result = None


# ---------------------------------------------------------------- host math
def _lstm_scan_np(x, Wih, Whh, bih, bhh):
    T = x.shape[0]
    WihT = np.ascontiguousarray(Wih.T)
    WhhT = np.ascontiguousarray(Whh.T)
    xg = x @ WihT + (bih + bhh)
    Hh = Whh.shape[1]
    h = np.zeros(Hh, np.float32)
    c = np.zeros(Hh, np.float32)
    hs = np.empty((T, Hh), np.float32)
    def sig(v):
        return 1.0 / (1.0 + np.exp(-v))
    for t in range(T):
        g = xg[t] + h @ WhhT
        i, f, gg, o = g[:Hh], g[Hh:2 * Hh], g[2 * Hh:3 * Hh], g[3 * Hh:]
        c = sig(f) * c + sig(i) * np.tanh(gg)
        h = sig(o) * np.tanh(c)
        hs[t] = h
    return hs


def _jax_noise():
    """eps1 [N1,H], eps2 [N2,N1,H] from jax.random key(42), computed on CPU."""
    try:
        import jax
        cpus = jax.devices("cpu")
        with jax.default_device(cpus[0]):
            k1, k2 = jax.random.split(jax.random.key(42))
            e1 = np.asarray(jax.random.normal(k1, (N1, H), np.float32))
            e2 = np.asarray(jax.random.normal(k2, (N2, N1, H), np.float32))
        return e1, e2
    except Exception:
        pass
    # Fallback: clean-env subprocess pinned to the CPU backend.
    import subprocess, tempfile
    code = (
        "import numpy as np, jax\n"
        "k1, k2 = jax.random.split(jax.random.key(42))\n"
        f"e1 = np.asarray(jax.random.normal(k1, ({N1}, {H}), np.float32))\n"
        f"e2 = np.asarray(jax.random.normal(k2, ({N2}, {N1}, {H}), np.float32))\n"
        "import sys; np.savez(sys.argv[1], e1=e1, e2=e2)\n"
    )
    env = dict(os.environ)
    env["JAX_PLATFORMS"] = "cpu"
    with tempfile.TemporaryDirectory() as td:
        out = os.path.join(td, "noise.npz")
        subprocess.run([sys.executable, "-c", code, out], check=True, env=env)
        z = np.load(out)
        return z["e1"], z["e2"]


# ---------------------------------------------------------------- device prog
def _build_program(stage=5):
    nc = bacc.Bacc("TRN2", target_bir_lowering=False, debug=False,
                   num_devices=NCORES)

    def din(name, shape, dt):
        return nc.dram_tensor(name, shape, dt, kind="ExternalInput").ap()

    def dout(name, shape, dt):
        return nc.dram_tensor(name, shape, dt, kind="ExternalOutput").ap()

    husw_d = din("husw", [128, 8, 1152], BF16)   # hT | UWT | SWT per k-chunk
    UbSb_d = din("UbSb", [128, 8], F32)
    eps_d = din("epsT", [128, 4, R], BF16)
    sel_d = din("selT", [128, 4, NSEL], FP8)
    fgW_d = din("fgWT", [2, 128, 2, 2 * VSH], FP8)   # k-pair interleaved
    fgb_d = din("fgb", [128, 2 * VSH], BF16)          # replicated bias rows

    uT_o = dout("uT", [4, 128, 128], F32)
    sT_o = dout("sT", [4, 128, 128], F32)
    sel1_o = dout("sel1", [128, 128], F32)
    sel2_o = dout("sel2", [16, 2048], F32)
    stats_o = dout("stats", [128, RT * 2], F32)

    from contextlib import ExitStack
    with tile.TileContext(nc) as tc, ExitStack() as stack:
        con = stack.enter_context(tc.tile_pool(name="con", bufs=1))
        husw_t = con.tile([128, 8, 1152], BF16, tag="husw", name="husw")
        hT = [husw_t[:, k, 0:128] for k in range(8)]
        UWT = [husw_t[:, k, 128:640] for k in range(8)]
        SWT = [husw_t[:, k, 640:1152] for k in range(8)]
        ubsb_t = con.tile([128, 8], F32, tag="ubsb", name="ubsb")
        Ub = [ubsb_t[:, m:m + 1] for m in range(4)]
        Sb = [ubsb_t[:, 4 + m:5 + m] for m in range(4)]
        eps_t = con.tile([128, 4, R], BF16, tag="eps", name="eps")
        epsT = [eps_t[:, c] for c in range(4)]
        sel_t = con.tile([128, 4, NSEL], FP8, tag="sel", name="sel")
        selT = [sel_t[:, c] for c in range(4)]
        fgWp = [con.tile([128, 2, 2 * VSH], FP8, tag=f"fgW{p}", name=f"fgW{p}")
                for p in range(2)]
        fgb = con.tile([128, 2 * VSH], BF16, tag="fgb")

        # Few, large DMAs on the two hardware-DGE queues; each fans out
        # across all 16 SDMA engines. Most-urgent tensors first per queue.
        nc.sync.dma_start(husw_t[:], husw_d[:])
        nc.scalar.dma_start(eps_t[:], eps_d[:])
        nc.sync.dma_start(ubsb_t[:], UbSb_d[:])
        nc.scalar.dma_start(sel_t[:], sel_d[:])
        nc.sync.dma_start(fgWp[0][:], fgW_d[0])
        nc.scalar.dma_start(fgWp[1][:], fgW_d[1])
        nc.sync.dma_start(fgb[:, :VSH], fgb_d[:, :VSH])
        nc.scalar.dma_start(fgb[:, VSH:], fgb_d[:, VSH:])

        work = stack.enter_context(tc.tile_pool(name="work", bufs=1))
        uTb = [work.tile([128, 128], BF16, tag=f"uTb{m}", name=f"uTb{m}") for m in range(4)]
        sTb = [work.tile([128, 128], BF16, tag=f"sTb{m}", name=f"sTb{m}") for m in range(4)]
        zTp = [work.tile([128, 2, R], FP8, tag=f"zTp{p}", name=f"zTp{p}") for p in range(2)]
        stats = work.tile([128, RT * 2], F32, tag="stats")

        def zch(c):          # z chunk c as [128, cols...] AP
            return zTp[c // 2][:, c % 2]

        # ---- phase 1: uT, sT  (uT = UW @ hT + Ub ; sT = softplus(SW @ hT + Sb))
        with tc.tile_pool(name="ps_small", bufs=2, space="PSUM") as ps_small, \
             tc.tile_pool(name="scr_small", bufs=2) as scr_small:
            for m in range(4 if stage >= 1 else 0):
                pu = ps_small.tile([128, 128], F32, tag="pus")
                for k in range(8):
                    nc.tensor.matmul(pu[:], UWT[k][:, m * 128:(m + 1) * 128],
                                     hT[k][:], start=(k == 0), stop=(k == 7))
                u32 = scr_small.tile([128, 128], F32, tag="u32")
                nc.vector.tensor_scalar_add(u32[:], pu[:], Ub[m][:])
                nc.vector.tensor_copy(uTb[m][:], u32[:])
                nc.sync.dma_start(uT_o[m], u32[:])

                psv = ps_small.tile([128, 128], F32, tag="pus")
                for k in range(8):
                    nc.tensor.matmul(psv[:], SWT[k][:, m * 128:(m + 1) * 128],
                                     hT[k][:], start=(k == 0), stop=(k == 7))
                es = scr_small.tile([128, 128], F32, tag="es")
                nc.scalar.activation(es[:], psv[:], AF.Exp, bias=Sb[m][:])
                s32 = scr_small.tile([128, 128], F32, tag="s32")
                nc.scalar.activation(s32[:], es[:], AF.Ln, bias=1.0)
                nc.vector.tensor_copy(sTb[m][:], s32[:])
                nc.sync.dma_start(sT_o[m], s32[:])

            # ---- phase 2: zT = u + s * eps  (broadcast over the 17 variants)
            NV = N2 + 1
            for c in range(4 if stage >= 2 else 0):
                tmp = scr_small.tile([128, R], BF16, tag="ztmp")
                nc.vector.tensor_mul(
                    tmp[:].rearrange("p (v t) -> p v t", v=NV),
                    epsT[c][:].rearrange("p (v t) -> p v t", v=NV),
                    sTb[c][:, None, :].broadcast_to([128, NV, 128]))
                nc.vector.tensor_add(
                    zch(c).rearrange("p (v t) -> p v t", v=NV),
                    tmp[:].rearrange("p (v t) -> p v t", v=NV),
                    uTb[c][:, None, :].broadcast_to([128, NV, 128]))

            # ---- phase 3: gathered-logit dot products
            if stage == 2:
                zc = scr_small.tile([128, 128], F32, tag="zc")
                nc.vector.tensor_copy(zc[:], zch(0)[:, 0:128])
                nc.sync.dma_start(sel1_o[:], zc[:])
            if stage >= 3:
                p1 = ps_small.tile([128, 128], F32, tag="p1", bufs=1)
                for k in range(4):
                    nc.tensor.matmul(p1[:], selT[k][:, 0:N1], zch(k)[:, 0:N1],
                                     start=(k == 0), stop=(k == 3))
                s1 = scr_small.tile([128, 128], F32, tag="s1")
                nc.vector.tensor_copy(s1[:], p1[:])
                nc.sync.dma_start(sel1_o[:], s1[:])

                p2 = ps_small.tile([16, 2048], F32, tag="p2", bufs=1)
                for k in range(4):
                    for nb in range(4):
                        nc.tensor.matmul(
                            p2[:, nb * 512:(nb + 1) * 512],
                            selT[k][:, N1:NSEL],
                            zch(k)[:, N1 + nb * 512:N1 + (nb + 1) * 512],
                            start=(k == 0), stop=(k == 3))
                s2 = scr_small.tile([16, 2048], F32, tag="s2")
                nc.vector.tensor_copy(s2[:], p2[:])
                nc.sync.dma_start(sel2_o[:], s2[:])

        # ---- phase 4: vocab-sharded logits + fused log-softmax partials
        with tc.tile_pool(name="ps_big", bufs=4, space="PSUM") as ps_big, \
             tc.tile_pool(name="scr_big", bufs=4) as scr_big:
            nrt = RT if stage >= 5 else (1 if stage == 4 else 0)
            for rt in range(nrt):
                vbase = 0 if rt == 0 else VSH   # z1 rows -> fW, z2 rows -> gW
                badd = None
                for vq in range(4):
                    w0 = vbase + vq * 1024
                    wid = VQ[vq]
                    ps = ps_big.tile([128, 1024], F32, tag="ps")
                    for p in range(2):
                        for s0 in range(0, wid, 512):
                            w = min(512, wid - s0)
                            nc.tensor.matmul(
                                ps[:, s0:s0 + w],
                                zTp[p][:, :, rt * 128:(rt + 1) * 128],
                                fgWp[p][:, :, w0 + s0:w0 + s0 + w],
                                start=(p == 0), stop=(p == 1),
                                perf_mode=PM.DoubleRow)
                    # bias add on DVE (psum f32 + bf16 row-replica -> bf16),
                    # pairs of quarters share one scratch for a wide exp op
                    half = vq // 2
                    if vq % 2 == 0:
                        badd = scr_big.tile([128, 2048], BF16, tag="badd")
                    nc.vector.tensor_add(
                        badd[:, vq % 2 * 1024:vq % 2 * 1024 + wid],
                        ps[:, :wid], fgb[:, w0:w0 + wid])
                    if vq % 2 == 1:
                        # |logits| <= ~10, exp cannot overflow fp32: skip
                        # max-stabilization, fuse exp + row-sum in one ACT op.
                        hw_ = 1024 + wid
                        col = rt * 2 + half
                        ex = scr_big.tile([128, 2048], BF16, tag="ex")
                        nc.scalar.activation(ex[:, :hw_], badd[:, :hw_],
                                             AF.Exp,
                                             accum_out=stats[:, col:col + 1])
            if nrt:
                nc.sync.dma_start(stats_o[:, :nrt * 2], stats[:, :nrt * 2])

    nc.compile()
    return nc


# ---------------------------------------------------------------- entry point
def _host_prep(inputs):
    gi = lambda n: np.asarray(inputs[n])
    words_l1 = gi("words_l1").astype(np.int64)
    words_l2 = gi("words_l2").astype(np.int64)
    emb = gi("emb").astype(np.float32)
    fW = gi("fW").astype(np.float32)
    fb = gi("fb").astype(np.float32)
    gW = gi("gW").astype(np.float32)
    gb = gi("gb").astype(np.float32)

    # host: embedding gather + sequential LSTM scans + PRNG noise
    x = emb[words_l1]
    hf = _lstm_scan_np(x, gi("Wih_f").astype(np.float32),
                       gi("Whh_f").astype(np.float32),
                       gi("bih_f").astype(np.float32),
                       gi("bhh_f").astype(np.float32))
    hb = _lstm_scan_np(x[::-1], gi("Wih_b").astype(np.float32),
                       gi("Whh_b").astype(np.float32),
                       gi("bih_b").astype(np.float32),
                       gi("bhh_b").astype(np.float32))[::-1]
    hcat = np.concatenate([hf, hb], axis=1)          # [N1, 2H]
    e1, e2 = _jax_noise()

    hT8 = np.ascontiguousarray(hcat.T).reshape(8, 128, 128)
    UW = gi("UW").astype(np.float32)
    SW = gi("SW").astype(np.float32)
    UWT = np.ascontiguousarray(UW.T).reshape(8, 128, 512)
    SWT = np.ascontiguousarray(SW.T).reshape(8, 128, 512)
    husw = np.ascontiguousarray(
        np.concatenate([hT8, UWT, SWT], axis=2).transpose(1, 0, 2)).astype(BF)
    UbSb = np.concatenate([gi("Ub"), gi("Sb")]).astype(np.float32)
    UbSb = np.ascontiguousarray(UbSb.reshape(8, 128).T)        # [128, 8]
    eps_all = np.concatenate([e1[None], e2], axis=0)          # [17, N1, H]
    epsT = np.ascontiguousarray(eps_all.transpose(2, 0, 1).reshape(H, R)
                                .reshape(4, 128, R).transpose(1, 0, 2)).astype(BF)
    selw = np.concatenate([fW[words_l1], gW[words_l2]], axis=0)  # [144, H]
    selT = np.ascontiguousarray(selw.T.reshape(4, 128, NSEL)
                                .transpose(1, 0, 2)).astype(E4)

    fWT = np.ascontiguousarray(fW.T)   # [H, V]
    gWT = np.ascontiguousarray(gW.T)

    shared = {"husw": husw, "UbSb": UbSb, "epsT": epsT, "selT": selT}
    in_maps = []
    for c in range(NCORES):
        sl = slice(c * VSH, (c + 1) * VSH)
        fgWT = np.concatenate([fWT[:, sl], gWT[:, sl]], axis=1)   # [H, 8000]
        # k-pair interleave: [2 pairs, 128 Ki, 2 planes, 8000]
        fgWT = np.ascontiguousarray(
            fgWT.reshape(2, 2, 128, 2 * VSH).transpose(0, 2, 1, 3)).astype(E4)
        fgb1 = np.concatenate([fb[sl], gb[sl]]).astype(np.float32)
        fgbr = np.ascontiguousarray(
            np.broadcast_to(fgb1, (128, 2 * VSH))).astype(BF)
        in_maps.append({**shared, "fgWT": fgWT, "fgb": fgbr})
    return in_maps, {"fb": fb, "gb": gb, "words_l1": words_l1,
                     "words_l2": words_l2}


def _combine(results, aux):
    fb, gb = aux["fb"], aux["gb"]
    words_l1, words_l2 = aux["words_l1"], aux["words_l2"]
    r0 = results[0]

    u = r0["uT"].reshape(H, N1).T.astype(np.float64)
    s = r0["sT"].reshape(H, N1).T.astype(np.float64)
    kl = 0.5 * (np.sum(s * s) + np.sum(u * u) - u.size - 2.0 * np.sum(np.log(s)))

    se = np.stack([results[c]["stats"].reshape(128, RT, 2)
                   for c in range(len(results))])       # [8, 128, RT, 2]
    S = se.astype(np.float64).sum(axis=(0, 3))          # [128, RT]
    lse = np.log(S).T.reshape(R)                        # row r = v*128 + t

    l1 = np.diag(r0["sel1"]).astype(np.float64) + fb[words_l1]
    term1 = np.sum(l1 - lse[:N1])
    j = np.arange(N2)
    l2 = r0["sel2"][j[:, None], j[:, None] * 128 + np.arange(N1)[None, :]]
    l2 = l2.astype(np.float64) + gb[words_l2][:, None]
    term2 = np.sum(l2 - lse[N1:].reshape(N2, N1)) / N2

    return np.asarray(-kl + term1 + term2, dtype=np.float32)


def kernel(**inputs):
    in_maps, aux = _host_prep(inputs)
    if "prog" not in _prog_cache:
        _prog_cache["prog"] = _build_program()
    nc = _prog_cache["prog"]

    res = run_bass_kernel_spmd(nc, in_maps, list(range(NCORES)))
    global last_result
    last_result = res
    return _combine(res.results, aux)
